# revision 1
# baseline (speedup 1.0000x reference)
"""Trainium2 Bass kernel for BERT factorized attention.

Reference math (per batch b, head h, S=4096, H=1024, NH=16, HD=64):
    q = x @ Wq + bq ; k = x @ Wk + bk ; v = x @ Wv + bv
    s_probs = softmax_S(qT_head)            # [HD, S]
    c_probs = softmax_HD(k_head)            # [S, HD]
    s_ctx   = s_probs @ v_head              # [HD, HD]
    out     = c_probs @ s_ctx               # [S, HD]

Kernel strategy (one batch element per NeuronCore, 8 cores, no collectives):
  - PE-transpose X -> XT per 512-row stripe; projections via float32r
    (TF32-like) matmuls at full PE rate (or bf16 when proj_dt=bf16).
  - Softmax denominators come from augmented matmuls:
      phase2: psum[d, 0:65] = EQ_chunk.T @ [V | 1]   accumulated over S
      den:    psum[s, 0:2]  = EK_tile.T @ ones-blockdiag  (c-softmax denom)
    (exp without max-subtraction is safe: q,k ~ N(0,1)).
  - EKT (exp of K-transposed) spills to DRAM scratch between pass A and B.
  - Pass B: out_head = (EKT_chunk.T @ s_ctx) * recip(denom) via one
    broadcast tensor_tensor multiply per head-pair.
"""

import sys

sys.path.insert(0, "/opt/trn_rl_repo")

import contextlib
from contextlib import ExitStack

import numpy as np

import concourse.bass as bass
import concourse.mybir as mybir
import concourse.tile as tile
from concourse import bacc, bass_utils
from concourse.masks import make_identity

F32 = mybir.dt.float32
F32R = mybir.dt.float32r
BF16 = mybir.dt.bfloat16

B, S, H = 8, 4096, 1024
NH, HD = 16, 64
STRIPE = 512
CPS = STRIPE // 128  # chunks per stripe
KT = H // 128  # contraction tiles
NP = NH // 2  # head pairs

EXPF = mybir.ActivationFunctionType.Exp
COPYF = mybir.ActivationFunctionType.Copy


def _bcast(ap_2d, n):
    """[p, c] AP -> [p, c, n] with step-0 broadcast on the last dim."""
    return bass.AP(
        tensor=ap_2d.tensor,
        offset=ap_2d.offset,
        ap=[ap_2d.ap[0], ap_2d.ap[1], [0, n]],
    )


def build_kernel(seq_len=S, with_bias=False, loop_n=None, proj_dt=F32R):
    """Build + compile the single-core program (SPMD across 8 cores)."""
    s = seq_len
    n_stripes = s // STRIPE
    n_chunks = s // 128

    nc = bacc.Bacc("TRN2", target_bir_lowering=False, debug=False, num_devices=8)

    x_d = nc.dram_tensor("x", [s, H], F32, kind="ExternalInput").ap()
    m_d = nc.dram_tensor("mask", [s], F32, kind="ExternalInput").ap()
    wq_d = nc.dram_tensor("wq", [H, H], F32, kind="ExternalInput").ap()
    wk_d = nc.dram_tensor("wk", [H, H], F32, kind="ExternalInput").ap()
    wv_d = nc.dram_tensor("wv", [H, H], F32, kind="ExternalInput").ap()
    if with_bias:
        bq_d = nc.dram_tensor("bq", [H], F32, kind="ExternalInput").ap()
        bk_d = nc.dram_tensor("bk", [H], F32, kind="ExternalInput").ap()
        bv_d = nc.dram_tensor("bv", [H], F32, kind="ExternalInput").ap()
    out_d = nc.dram_tensor("out", [s, H], F32, kind="ExternalOutput").ap()
    ekt_ds = [
        nc.dram_tensor(f"ekts{i}", [KT, 128, STRIPE], BF16, kind="Internal").ap()
        for i in range(n_stripes)
    ]

    with tile.TileContext(nc) as tc:
        with ExitStack() as ctx:
            singles = ctx.enter_context(tc.tile_pool(name="singles", bufs=1))
            xpool = ctx.enter_context(tc.tile_pool(name="xpool", bufs=2))
            xtpool = ctx.enter_context(tc.tile_pool(name="xtpool", bufs=2))
            eqpool = ctx.enter_context(tc.tile_pool(name="eqpool", bufs=3))
            vapool = ctx.enter_context(tc.tile_pool(name="vapool", bufs=3))
            ekpool = ctx.enter_context(tc.tile_pool(name="ekpool", bufs=3))
            ecpool = ctx.enter_context(tc.tile_pool(name="ecpool", bufs=2))
            opool = ctx.enter_context(tc.tile_pool(name="opool", bufs=3))
            small = ctx.enter_context(tc.tile_pool(name="small", bufs=4))
            # PSUM (8 banks): tp 2 + proj 2 + kt 2 + p2 2 (shared with den)
            tp = ctx.enter_context(tc.tile_pool(name="tp", bufs=2, space="PSUM"))
            proj = ctx.enter_context(tc.tile_pool(name="proj", bufs=2, space="PSUM"))
            ktp = ctx.enter_context(tc.tile_pool(name="ktp", bufs=2, space="PSUM"))
            p2p = ctx.enter_context(tc.tile_pool(name="p2p", bufs=2, space="PSUM"))
            dpp = p2p

            ident = singles.tile([128, 128], F32)
            make_identity(nc, ident)

            mask_sb = singles.tile([128, n_chunks], F32)
            nc.gpsimd.dma_start(out=mask_sb, in_=m_d.rearrange("(c p) -> p c", p=128))

            # weights: DMA fp32 staging chunks, round to proj dtype
            w_r = {}
            for name, wd in (("wq", wq_d), ("wv", wv_d), ("wk", wk_d)):
                wr = singles.tile([128, KT, H], proj_dt, tag=f"{name}_r")
                w_r[name] = wr
                for k in range(KT):
                    st = opool.tile([128, H], F32, tag="ob")
                    nc.gpsimd.dma_start(out=st, in_=wd[k * 128 : (k + 1) * 128, :])
                    nc.scalar.activation(wr[:, k, :], st, COPYF)
            wq_r, wk_r, wv_r = w_r["wq"], w_r["wk"], w_r["wv"]

            if with_bias:
                bqb = singles.tile([128, H], F32)
                bvb = singles.tile([128, H], F32)
                for bt, bd in ((bqb, bq_d), (bvb, bv_d)):
                    src = bass.AP(
                        tensor=bd.tensor, offset=bd.offset, ap=[[0, 128], bd.ap[0]]
                    )
                    nc.sync.dma_start(out=bt, in_=src)
                bkc = singles.tile([128, KT], F32)
                nc.sync.dma_start(out=bkc, in_=bk_d.rearrange("(t p) -> p t", p=128))

            acc = singles.tile([128, NP, 130], F32)
            sctf = singles.tile([128, NP, 128], F32)
            sctx = singles.tile([128, NP, 128], BF16)
            sctl = singles.tile([128, NP, 128], BF16)
            denoms = singles.tile([128, n_chunks, 16], F32)
            # f32r constants (memset of f32r is rejected by walrus; use
            # fp32->f32r tensor_copy from fp32 constant tiles)
            ones16 = singles.tile([128, 16, 1], F32)
            nc.vector.memset(ones16, 1.0)
            z128 = singles.tile([128, 128], F32)
            nc.vector.memset(z128, 0.0)
            ones2 = singles.tile([128, 2], BF16)
            nc.vector.tensor_copy(ones2, z128[:, 0:2])
            nc.vector.tensor_copy(ones2[0:64, 0:1], ones16[0:64, 0, :])
            nc.vector.tensor_copy(ones2[64:128, 1:2], ones16[64:128, 0, :])

            loop_cm = tc.For_i(0, loop_n, 1) if loop_n else contextlib.nullcontext()
            with loop_cm:
                nc.vector.memset(acc, 0.0)

                # ---------------- PASS A ----------------
                for st_i in range(n_stripes):
                    s0 = st_i * STRIPE
                    xt = xtpool.tile([128, KT, STRIPE], proj_dt)
                    for c in range(CPS):
                        xc = xpool.tile([128, H], F32)
                        nc.sync.dma_start(
                            out=xc, in_=x_d[s0 + c * 128 : s0 + (c + 1) * 128, :]
                        )
                        for g in range(KT // 4):
                            pt = tp.tile([128, 4, 128], F32)
                            for kk in range(4):
                                k = g * 4 + kk
                                nc.tensor.transpose(
                                    pt[:, kk, :],
                                    xc[:, k * 128 : (k + 1) * 128],
                                    ident,
                                )
                            nc.vector.tensor_copy(
                                xt[:, g * 4 : (g + 1) * 4, c * 128 : (c + 1) * 128],
                                pt,
                            )

                    # Q,V projections + phase 2 in chunk pairs
                    for cp in range(CPS // 2):
                        eqs, vas = [], []
                        for c in (2 * cp, 2 * cp + 1):
                            sc = st_i * CPS + c
                            cs = slice(c * 128, (c + 1) * 128)
                            eqc = eqpool.tile([128, H], F32R, tag="eq")
                            vac = vapool.tile([128, NH, 65], F32R, tag="va")
                            eqs.append(eqc)
                            vas.append(vac)
                            mb = mask_sb[:, sc : sc + 1]
                            for half in range(2):
                                hs = slice(half * 512, (half + 1) * 512)
                                pq = proj.tile([128, 512], F32, tag="proj")
                                for k in range(KT):
                                    nc.tensor.matmul(
                                        pq,
                                        xt[:, k, cs],
                                        wq_r[:, k, hs],
                                        start=k == 0,
                                        stop=k == KT - 1,
                                    )
                                if with_bias:
                                    nc.vector.tensor_add(pq, pq, bqb[:, hs])
                                nc.scalar.activation(eqc[:, hs], pq, EXPF, bias=mb)
                            for half in range(2):
                                hs = slice(half * 512, (half + 1) * 512)
                                pv = proj.tile([128, 512], F32, tag="proj")
                                for k in range(KT):
                                    nc.tensor.matmul(
                                        pv,
                                        xt[:, k, cs],
                                        wv_r[:, k, hs],
                                        start=k == 0,
                                        stop=k == KT - 1,
                                    )
                                dst = vac[:, half * 8 : (half + 1) * 8, 0:64]
                                src = pv[:].rearrange("p (h e) -> p h e", e=64)
                                if with_bias:
                                    nc.vector.tensor_add(
                                        dst,
                                        src,
                                        bvb[:, hs].rearrange(
                                            "p (h e) -> p h e", e=64
                                        ),
                                    )
                                else:
                                    nc.scalar.activation(dst, src, COPYF)
                            nc.vector.tensor_copy(vac[:, :, 64:65], ones16)

                        for hp in range(NP):
                            p2 = p2p.tile([128, 130], F32, tag="p2")
                            for j in range(2):
                                nc.tensor.matmul(
                                    p2,
                                    eqs[j][:, hp * 128 : (hp + 1) * 128],
                                    vas[j][:, hp * 2 : hp * 2 + 2, :],
                                    start=j == 0,
                                    stop=j == 1,
                                )
                            nc.vector.tensor_add(acc[:, hp, :], acc[:, hp, :], p2)

                    # K-transposed projection -> exp -> denoms + DRAM scratch
                    for t in range(KT):
                        pk = ktp.tile([128, 512], F32, tag="pk")
                        for k in range(KT):
                            nc.tensor.matmul(
                                pk,
                                wk_r[:, k, t * 128 : (t + 1) * 128],
                                xt[:, k, :],
                                start=k == 0,
                                stop=k == KT - 1,
                            )
                        ek = ekpool.tile([128, STRIPE], BF16)
                        if with_bias:
                            nc.scalar.activation(
                                ek, pk, EXPF, bias=bkc[:, t : t + 1]
                            )
                        else:
                            nc.scalar.activation(ek, pk, EXPF)
                        nc.gpsimd.dma_start(out=ekt_ds[st_i][t], in_=ek)
                        dps = dpp.tile([128, CPS, 2], F32, tag="p2")
                        for c in range(CPS):
                            nc.tensor.matmul(
                                dps[:, c, :],
                                ek[:, c * 128 : (c + 1) * 128],
                                ones2,
                                start=True,
                                stop=True,
                            )
                        nc.vector.tensor_copy(
                            denoms[
                                :, st_i * CPS : (st_i + 1) * CPS, 2 * t : 2 * t + 2
                            ],
                            dps,
                        )

                # ---------------- finalize s_ctx (bf16 hi/lo split) ------
                nc.vector.memset(sctf, 0.0)
                for hp in range(NP):
                    r0 = small.tile([64, 1], F32, tag="r0")
                    r1 = small.tile([64, 1], F32, tag="r1")
                    nc.vector.reciprocal(r0, acc[0:64, hp, 64:65])
                    nc.vector.reciprocal(r1, acc[64:128, hp, 129:130])
                    nc.vector.tensor_scalar_mul(
                        sctf[0:64, hp, 0:64], acc[0:64, hp, 0:64], r0
                    )
                    nc.vector.tensor_scalar_mul(
                        sctf[64:128, hp, 64:128], acc[64:128, hp, 65:129], r1
                    )
                nc.vector.tensor_copy(sctx, sctf)
                nc.vector.tensor_tensor(
                    out=sctl, in0=sctf, in1=sctx, op=mybir.AluOpType.subtract
                )

                # ---------------- PASS B ----------------
                for cc in range(n_chunks):
                    if cc % CPS == 0:
                        ec = ecpool.tile([128, KT, STRIPE], BF16)
                        for t in range(KT):
                            nc.sync.dma_start(out=ec[:, t, :], in_=ekt_ds[cc // CPS][t])
                    co = cc % CPS
                    r16 = small.tile([128, 16], F32, tag="r16")
                    nc.vector.reciprocal(r16, denoms[:, cc, :])
                    ob = opool.tile([128, H], F32)
                    for hp in range(NP):
                        if hp % 2 == 0:
                            p3 = proj.tile([128, 128], F32, tag="proj")
                        else:
                            p3 = ktp.tile([128, 128], F32, tag="pk")
                        nc.tensor.matmul(
                            p3,
                            ec[:, hp, co * 128 : (co + 1) * 128],
                            sctx[:, hp, :],
                            start=True,
                            stop=False,
                        )
                        nc.tensor.matmul(
                            p3,
                            ec[:, hp, co * 128 : (co + 1) * 128],
                            sctl[:, hp, :],
                            start=False,
                            stop=True,
                        )
                        dst = ob[:, hp * 128 : (hp + 1) * 128]
                        if hp % 2 == 0:
                            nc.vector.tensor_copy(dst, p3)
                        else:
                            nc.scalar.activation(dst, p3, COPYF)
                    obv = ob[:].rearrange("p (h e) -> p h e", e=64)
                    nc.vector.tensor_tensor(
                        out=obv, in0=obv, in1=_bcast(r16, 64),
                        op=mybir.AluOpType.mult,
                    )
                    nc.gpsimd.dma_start(
                        out=out_d[cc * 128 : (cc + 1) * 128, :], in_=ob
                    )

    nc.compile()
    return nc


_CACHE = {}


def _get_nc(seq_len, with_bias):
    key = (seq_len, with_bias)
    if key not in _CACHE:
        _CACHE[key] = build_kernel(seq_len, with_bias)
    return _CACHE[key]


def kernel(hidden_states, attention_mask, Wq, bq, Wk, bk, Wv, bv):
    hidden_states = np.asarray(hidden_states, dtype=np.float32)
    attention_mask = np.asarray(attention_mask, dtype=np.float32)
    Wq = np.asarray(Wq, dtype=np.float32)
    Wk = np.asarray(Wk, dtype=np.float32)
    Wv = np.asarray(Wv, dtype=np.float32)
    bq = np.asarray(bq, dtype=np.float32)
    bk = np.asarray(bk, dtype=np.float32)
    bv = np.asarray(bv, dtype=np.float32)
    b, s, h = hidden_states.shape
    with_bias = bool(bq.any() or bk.any() or bv.any())
    nc = _get_nc(s, with_bias)

    mask = attention_mask.reshape(b, s)
    in_maps = []
    for i in range(b):
        m = {
            "x": np.ascontiguousarray(hidden_states[i]),
            "mask": np.ascontiguousarray(mask[i]),
            "wq": Wq,
            "wk": Wk,
            "wv": Wv,
        }
        if with_bias:
            m.update({"bq": bq, "bk": bk, "bv": bv})
        in_maps.append(m)

    res = bass_utils.run_bass_kernel_spmd(nc, in_maps, core_ids=list(range(b)))
    return np.stack([res.results[i]["out"] for i in range(b)], axis=0)



# revision 15
# speedup vs baseline: 1.6357x; 1.6357x over previous
"""Trainium2 Bass kernel for BERT factorized attention.

Reference math (per batch b, head h, S=4096, H=1024, NH=16, HD=64):
    q = x @ Wq + bq ; k = x @ Wk + bk ; v = x @ Wv + bv
    s_probs = softmax_S(qT_head)            # [HD, S]
    c_probs = softmax_HD(k_head)            # [S, HD]
    s_ctx   = s_probs @ v_head              # [HD, HD]
    out     = c_probs @ s_ctx               # [S, HD]

Kernel strategy (one batch element per NeuronCore, 8 cores, no collectives):
  - X cast to fp8e4m3, PE-transposed per chunk; Q/K/V projections run as
    fp8 DoubleRow matmuls (256-deep contraction, 2x PE rate). Weights are
    pre-scaled by 32 into fp8; the scale is undone inside the exp
    activation (scale=1/32) / V evacuation.
  - softmax-over-S denominators come from an augmented ones column in the
    V operand of phase 2 (s_ctx accumulation); phase 2 accumulates
    directly in PSUM across all 32 chunks (3 pinned banks, one
    accumulation group per bank).
  - EKT (exp of K-transposed, bf16) lives entirely in SBUF.
  - softmax-over-HD denominators come from two ones-columns appended to
    s_ctx in the pass-B matmul; out = p3 * recip(den) per head via
    fused tensor_scalar/activation-scale evacuations.
"""

import sys

sys.path.insert(0, "/opt/trn_rl_repo")

import contextlib
from contextlib import ExitStack

import numpy as np

import concourse.bass as bass
import concourse.mybir as mybir
import concourse.tile as tile
from concourse import bacc, bass_utils
from concourse.masks import make_identity

F32 = mybir.dt.float32
BF16 = mybir.dt.bfloat16
F8 = mybir.dt.float8e4

B, S, H = 8, 4096, 1024
NH, HD = 16, 64
STRIPE = 512
CPS = STRIPE // 128  # chunks per stripe
KT = H // 128  # contraction tiles
NP = NH // 2  # head pairs

WSCALE = 32.0
INV_WSCALE = 1.0 / WSCALE

EXPF = mybir.ActivationFunctionType.Exp
COPYF = mybir.ActivationFunctionType.Copy
DR = mybir.MatmulPerfMode.DoubleRow


def _bcast(ap_2d, n):
    """[p, c] AP -> [p, c, n] with step-0 broadcast on the last dim."""
    return bass.AP(
        tensor=ap_2d.tensor,
        offset=ap_2d.offset,
        ap=[ap_2d.ap[0], ap_2d.ap[1], [0, n]],
    )


def build_kernel(seq_len=S, with_bias=False, loop_n=None, proj_fp8=False,
                 use_lo=False, recip_fast=True, gps_cast=True):
    """Build + compile the single-core program (SPMD across 8 cores)."""
    s = seq_len
    n_stripes = s // STRIPE
    n_chunks = s // 128
    pdt = F8 if proj_fp8 else BF16
    psc = INV_WSCALE if proj_fp8 else 1.0

    nc = bacc.Bacc("TRN2", target_bir_lowering=False, debug=False, num_devices=8)

    x_d = nc.dram_tensor("x", [s, H], F32, kind="ExternalInput").ap()
    m_d = nc.dram_tensor("mask", [s], F32, kind="ExternalInput").ap()
    wq_d = nc.dram_tensor("wq", [H, H], F32, kind="ExternalInput").ap()
    wk_d = nc.dram_tensor("wk", [H, H], F32, kind="ExternalInput").ap()
    wv_d = nc.dram_tensor("wv", [H, H], F32, kind="ExternalInput").ap()
    if with_bias:
        bq_d = nc.dram_tensor("bq", [H], F32, kind="ExternalInput").ap()
        bk_d = nc.dram_tensor("bk", [H], F32, kind="ExternalInput").ap()
        bv_d = nc.dram_tensor("bv", [H], F32, kind="ExternalInput").ap()
    out_d = nc.dram_tensor("out", [s, H], F32, kind="ExternalOutput").ap()

    with tile.TileContext(nc) as tc:
        with ExitStack() as ctx:
            singles = ctx.enter_context(tc.tile_pool(name="singles", bufs=1))
            xpool = ctx.enter_context(tc.tile_pool(name="xpool", bufs=3))
            x8pool = ctx.enter_context(tc.tile_pool(name="x8pool", bufs=2))
            xtpool = ctx.enter_context(tc.tile_pool(name="xtpool", bufs=2))
            eqpool = ctx.enter_context(tc.tile_pool(name="eqpool", bufs=2))
            vapool = ctx.enter_context(tc.tile_pool(name="vapool", bufs=2))
            opool = ctx.enter_context(tc.tile_pool(name="opool", bufs=3))
            small = ctx.enter_context(tc.tile_pool(name="small", bufs=4))
            # PSUM (8 banks): tp 2 + proj 3 + p2 3
            tp = ctx.enter_context(tc.tile_pool(name="tp", bufs=2, space="PSUM"))
            proj = ctx.enter_context(tc.tile_pool(name="proj", bufs=3, space="PSUM"))
            p2p = ctx.enter_context(tc.tile_pool(name="p2p", bufs=1, space="PSUM"))

            ident = singles.tile([128, 128], F32)
            make_identity(nc, ident)
            identp = singles.tile([128, 128], pdt)
            nc.vector.tensor_copy(identp, ident)

            mask_sb = singles.tile([128, n_chunks], F32)
            nc.gpsimd.dma_start(out=mask_sb, in_=m_d.rearrange("(c p) -> p c", p=128))

            # weights: DMA fp32 staging chunks, scale+round to fp8 (or bf16)
            wsc = WSCALE if proj_fp8 else 1.0
            w_r = {}
            wstage = ctx.enter_context(tc.tile_pool(name="wstage", bufs=3))
            for name, wd in (("wq", wq_d), ("wv", wv_d), ("wk", wk_d)):
                wr = singles.tile([128, KT, H], pdt, tag=f"{name}_r", name=f"{name}_r")
                w_r[name] = wr
                for k in range(KT):
                    st = wstage.tile([128, H], F32, tag="wst", name="wst")
                    nc.gpsimd.dma_start(out=st, in_=wd[k * 128 : (k + 1) * 128, :])
                    if proj_fp8 or not gps_cast:
                        nc.scalar.activation(wr[:, k, :], st, COPYF, scale=wsc)
                    else:
                        nc.gpsimd.tensor_copy(wr[:, k, :], st)
            wq_r, wk_r, wv_r = w_r["wq"], w_r["wk"], w_r["wv"]

            if with_bias:
                # bq pre-scaled by wsc (added to scaled psum before exp);
                # bv added unscaled after the V evacuation descale.
                bqb = singles.tile([128, H], F32)
                bvb = singles.tile([128, H], F32)
                for bt, bd, bs in ((bqb, bq_d, wsc), (bvb, bv_d, 1.0)):
                    st = opool.tile([128, H], F32, tag="ob", name="bst")
                    src = bass.AP(
                        tensor=bd.tensor, offset=bd.offset, ap=[[0, 128], bd.ap[0]]
                    )
                    nc.sync.dma_start(out=st, in_=src)
                    nc.scalar.activation(bt, st, COPYF, scale=bs)
                bkc = singles.tile([128, KT], F32)
                nc.sync.dma_start(out=bkc, in_=bk_d.rearrange("(t p) -> p t", p=128))

            # EKT: exp(K^T) for the whole sequence, bf16, SBUF-resident
            ekt = singles.tile([128, KT, s], BF16)

            # s_ctx staging: [128, NP, 128]; cols 0:64 head-A sctx,
            # 64:128 head-B sctx (row blocks 0:64 / 64:128); off-diagonal
            # blocks stay zero.
            sctf = singles.tile([128, NP, 128], F32)
            sctx = singles.tile([128, NP, 128], BF16)
            if use_lo:
                sctl = singles.tile([128, NP, 128], BF16)
            nc.vector.memset(sctf, 0.0)

            # block-diagonal ones operand for the pass-B denominator matmuls
            # (partial-partition writes via tensor_copy from fp32 tiles;
            # partition-offset memset does not lower correctly)
            ones1 = singles.tile([128, 1], F32)
            nc.vector.memset(ones1, 1.0)
            z2 = singles.tile([128, 2], F32)
            nc.vector.memset(z2, 0.0)
            ones2 = singles.tile([128, 2], BF16)
            nc.vector.tensor_copy(ones2, z2)
            nc.vector.tensor_copy(ones2[0:64, 0:1], ones1[0:64, :])
            nc.vector.tensor_copy(ones2[64:128, 1:2], ones1[64:128, :])

            # phase-2 PSUM accumulators: 3 banks, head-pairs (0,1,2),
            # (3,4,5), (6,7) at 130-col offsets
            p2b = [
                p2p.tile([128, 390], F32, tag=f"p2b{i}", name=f"p2b{i}")
                for i in range(3)
            ]

            def p2ap(hp):
                return p2b[hp // 3][:, (hp % 3) * 130 : (hp % 3 + 1) * 130]

            P2_FIRST = {0, 3, 6}
            P2_LAST = {2, 5, 7}

            def emit_proj(pq, lhsT_tile, lhs_cs, w_tile, w_cs):
                """psum += xT_chunk.T @ W (or W.T @ xT for the K path)."""
                if proj_fp8:
                    for kp in range(KT // 2):
                        ks = slice(2 * kp, 2 * kp + 2)
                        nc.tensor.matmul(
                            pq,
                            lhsT_tile[:, ks, lhs_cs],
                            w_tile[:, ks, w_cs],
                            start=kp == 0,
                            stop=kp == KT // 2 - 1,
                            perf_mode=DR,
                        )
                else:
                    for k in range(KT):
                        nc.tensor.matmul(
                            pq,
                            lhsT_tile[:, k, lhs_cs],
                            w_tile[:, k, w_cs],
                            start=k == 0,
                            stop=k == KT - 1,
                        )

            loop_cm = tc.For_i(0, loop_n, 1) if loop_n else contextlib.nullcontext()
            with loop_cm:
                pend = None  # (eq_tile, va_tile, global_chunk) awaiting phase 2

                def emit_p2(eqc, vac, sc):
                    for hp in range(NP):
                        nc.tensor.matmul(
                            p2ap(hp),
                            eqc[:, hp * 128 : (hp + 1) * 128],
                            vac[:, hp * 2 : hp * 2 + 2, :],
                            start=(sc == 0 and hp in P2_FIRST),
                            stop=(sc == n_chunks - 1 and hp in P2_LAST),
                        )

                for st_i in range(n_stripes):
                    xt8 = xtpool.tile([128, KT, STRIPE], pdt, name="xt8")

                    def emit_t(c):
                        sc = st_i * CPS + c
                        xc = xpool.tile([128, H], F32, tag="xc", name="xc")
                        nc.sync.dma_start(
                            out=xc, in_=x_d[sc * 128 : (sc + 1) * 128, :]
                        )
                        xf8 = x8pool.tile([128, H], pdt, tag="xf8", name="xf8")
                        (nc.gpsimd if gps_cast else nc.vector).tensor_copy(xf8, xc)
                        for g in range(2):
                            pt = tp.tile([128, 4, 128], pdt, tag="pt", name="pt")
                            for j in range(4):
                                k = g * 4 + j
                                nc.tensor.transpose(
                                    pt[:, j, :],
                                    xf8[:, k * 128 : (k + 1) * 128],
                                    identp,
                                )
                            nc.vector.tensor_copy(
                                xt8[:, g * 4 : (g + 1) * 4, c * 128 : (c + 1) * 128],
                                pt,
                            )

                    emit_t(0)
                    for c in range(CPS):
                        sc = st_i * CPS + c
                        cs = slice(c * 128, (c + 1) * 128)
                        # Q projection -> exp -> EQ bf16
                        eqc = eqpool.tile([128, H], BF16, tag="eq", name="eq")
                        mb = mask_sb[:, sc : sc + 1]
                        for half in range(2):
                            hs = slice(half * 512, (half + 1) * 512)
                            pq = proj.tile([128, 512], F32, tag="proj", name="pq")
                            emit_proj(pq, xt8, cs, wq_r, hs)
                            if with_bias:
                                nc.vector.tensor_add(pq, pq, bqb[:, hs])
                            nc.scalar.activation(
                                eqc[:, hs], pq, EXPF, bias=mb, scale=psc
                            )
                        if c + 1 < CPS:
                            emit_t(c + 1)
                        # V projection -> descale -> VA bf16 (+ ones column)
                        vac = vapool.tile([128, NH, 65], BF16, tag="va", name="va")
                        for half in range(2):
                            hs = slice(half * 512, (half + 1) * 512)
                            pv = proj.tile([128, 512], F32, tag="proj", name="pv")
                            emit_proj(pv, xt8, cs, wv_r, hs)
                            dst = vac[:, half * 8 : (half + 1) * 8, 0:64]
                            src = pv[:].rearrange("p (h e) -> p h e", e=64)
                            if proj_fp8:
                                nc.vector.tensor_scalar_mul(dst, src, INV_WSCALE)
                            else:
                                nc.vector.tensor_copy(dst, src)
                            if with_bias:
                                nc.vector.tensor_add(
                                    dst,
                                    dst,
                                    bvb[:, hs].rearrange("p (h e) -> p h e", e=64),
                                )
                        nc.vector.memset(vac[:, :, 64:65], 1.0)
                        if pend is not None:
                            emit_p2(*pend)
                        pend = (eqc, vac, sc)

                    # K-transposed projection -> exp -> EKT (SBUF)
                    for t in range(KT):
                        ts_ = slice(t * 128, (t + 1) * 128)
                        pk = proj.tile([128, 512], F32, tag="proj", name="pk")
                        emit_proj(pk, wk_r, ts_, xt8, slice(0, STRIPE))
                        kb = {"bias": bkc[:, t : t + 1]} if with_bias else {}
                        nc.scalar.activation(
                            ekt[:, t, st_i * STRIPE : (st_i + 1) * STRIPE],
                            pk,
                            EXPF,
                            scale=psc,
                            **kb,
                        )

                # last pending phase 2
                emit_p2(*pend)
                pend = None

                # ---------------- finalize s_ctx ----------------
                for hp in range(NP):
                    pv2 = p2ap(hp)
                    r0 = small.tile([64, 1], F32, tag="r0", name="r0")
                    r1 = small.tile([64, 1], F32, tag="r1", name="r1")
                    nc.vector.reciprocal(r0, pv2[0:64, 64:65])
                    nc.vector.reciprocal(r1, pv2[64:128, 129:130])
                    nc.vector.tensor_scalar_mul(
                        sctf[0:64, hp, 0:64], pv2[0:64, 0:64], r0
                    )
                    nc.vector.tensor_scalar_mul(
                        sctf[64:128, hp, 64:128], pv2[64:128, 65:129], r1
                    )
                nc.vector.tensor_copy(sctx, sctf)
                if use_lo:
                    nc.vector.tensor_tensor(
                        out=sctl, in0=sctf, in1=sctx, op=mybir.AluOpType.subtract
                    )

                # ---------------- PASS B ----------------
                # Per chunk: 8 tiny matmuls EKT_hp.T @ ones2 fill a [128, 16]
                # denominator tile (one bank); main matmuls pack 4 head-pairs
                # per bank (4 x 128 cols); evacuation is one
                # reciprocal_approx_fast + two broadcast multiplies.
                for cc in range(n_chunks):
                    ob = opool.tile([128, H], F32, tag="ob", name="ob")
                    ccs = slice(cc * 128, (cc + 1) * 128)
                    pd = proj.tile([128, 512], F32, tag="proj", name="pd")
                    for hp in range(NP):
                        nc.tensor.matmul(
                            pd[:, hp * 2 : hp * 2 + 2],
                            ekt[:, hp, ccs],
                            ones2,
                            start=hp == 0,
                            stop=hp == NP - 1,
                        )
                    r16 = small.tile([128, 16], F32, tag="r16", name="r16")
                    recip = (nc.vector.reciprocal_approx_fast if recip_fast
                             else nc.vector.reciprocal)
                    recip(r16, pd[:, 0:16])
                    for g in range(2):
                        p3 = proj.tile([128, 512], F32, tag="proj", name="p3")
                        for i in range(4):
                            hp = g * 4 + i
                            nc.tensor.matmul(
                                p3[:, i * 128 : (i + 1) * 128],
                                ekt[:, hp, ccs],
                                sctx[:, hp, :],
                                start=(i == 0),
                                stop=(i == 3 and not use_lo),
                            )
                        if use_lo:
                            for i in range(4):
                                hp = g * 4 + i
                                nc.tensor.matmul(
                                    p3[:, i * 128 : (i + 1) * 128],
                                    ekt[:, hp, ccs],
                                    sctl[:, hp, :],
                                    start=False,
                                    stop=(i == 3),
                                )
                        obv = ob[:, g * 512 : (g + 1) * 512].rearrange(
                            "p (h e) -> p h e", e=64
                        )
                        p3v = p3[:].rearrange("p (h e) -> p h e", e=64)
                        nc.vector.tensor_tensor(
                            out=obv,
                            in0=p3v,
                            in1=_bcast(r16[:, g * 8 : (g + 1) * 8], 64),
                            op=mybir.AluOpType.mult,
                        )
                    eng = nc.gpsimd if cc % 2 == 0 else nc.sync
                    eng.dma_start(out=out_d[ccs, :], in_=ob)

    nc.compile()
    return nc


_CACHE = {}


def _get_nc(seq_len, with_bias):
    key = (seq_len, with_bias)
    if key not in _CACHE:
        _CACHE[key] = build_kernel(seq_len, with_bias)
    return _CACHE[key]


def kernel(hidden_states, attention_mask, Wq, bq, Wk, bk, Wv, bv):
    hidden_states = np.asarray(hidden_states, dtype=np.float32)
    attention_mask = np.asarray(attention_mask, dtype=np.float32)
    Wq = np.asarray(Wq, dtype=np.float32)
    Wk = np.asarray(Wk, dtype=np.float32)
    Wv = np.asarray(Wv, dtype=np.float32)
    bq = np.asarray(bq, dtype=np.float32)
    bk = np.asarray(bk, dtype=np.float32)
    bv = np.asarray(bv, dtype=np.float32)
    b, s, h = hidden_states.shape
    with_bias = bool(bq.any() or bk.any() or bv.any())
    nc = _get_nc(s, with_bias)

    mask = attention_mask.reshape(b, s)
    in_maps = []
    for i in range(b):
        m = {
            "x": np.ascontiguousarray(hidden_states[i]),
            "mask": np.ascontiguousarray(mask[i]),
            "wq": Wq,
            "wk": Wk,
            "wv": Wv,
        }
        if with_bias:
            m.update({"bq": bq, "bk": bk, "bv": bv})
        in_maps.append(m)

    res = bass_utils.run_bass_kernel_spmd(nc, in_maps, core_ids=list(range(b)))
    return np.stack([res.results[i]["out"] for i in range(b)], axis=0)


# revision 16
# speedup vs baseline: 1.7178x; 1.0502x over previous
"""Trainium2 Bass kernel for BERT factorized attention.

Reference math (per batch b, head h, S=4096, H=1024, NH=16, HD=64):
    q = x @ Wq + bq ; k = x @ Wk + bk ; v = x @ Wv + bv
    s_probs = softmax_S(qT_head)            # [HD, S]
    c_probs = softmax_HD(k_head)            # [S, HD]
    s_ctx   = s_probs @ v_head              # [HD, HD]
    out     = c_probs @ s_ctx               # [S, HD]

Kernel strategy (one batch element per NeuronCore, 8 cores, no collectives):
  - X cast to fp8e4m3, PE-transposed per chunk; Q/K/V projections run as
    fp8 DoubleRow matmuls (256-deep contraction, 2x PE rate). Weights are
    pre-scaled by 32 into fp8; the scale is undone inside the exp
    activation (scale=1/32) / V evacuation.
  - softmax-over-S denominators come from an augmented ones column in the
    V operand of phase 2 (s_ctx accumulation); phase 2 accumulates
    directly in PSUM across all 32 chunks (3 pinned banks, one
    accumulation group per bank).
  - EKT (exp of K-transposed, bf16) lives entirely in SBUF.
  - softmax-over-HD denominators come from two ones-columns appended to
    s_ctx in the pass-B matmul; out = p3 * recip(den) per head via
    fused tensor_scalar/activation-scale evacuations.
"""

import sys

sys.path.insert(0, "/opt/trn_rl_repo")

import contextlib
from contextlib import ExitStack

import numpy as np

import concourse.bass as bass
import concourse.mybir as mybir
import concourse.tile as tile
from concourse import bacc, bass_utils
from concourse.masks import make_identity

F32 = mybir.dt.float32
BF16 = mybir.dt.bfloat16
F8 = mybir.dt.float8e4

B, S, H = 8, 4096, 1024
NH, HD = 16, 64
STRIPE = 512
CPS = STRIPE // 128  # chunks per stripe
KT = H // 128  # contraction tiles
NP = NH // 2  # head pairs

WSCALE = 32.0
INV_WSCALE = 1.0 / WSCALE

EXPF = mybir.ActivationFunctionType.Exp
COPYF = mybir.ActivationFunctionType.Copy
DR = mybir.MatmulPerfMode.DoubleRow


def _bcast(ap_2d, n):
    """[p, c] AP -> [p, c, n] with step-0 broadcast on the last dim."""
    return bass.AP(
        tensor=ap_2d.tensor,
        offset=ap_2d.offset,
        ap=[ap_2d.ap[0], ap_2d.ap[1], [0, n]],
    )


def build_kernel(seq_len=S, with_bias=False, loop_n=None, proj_fp8=False,
                 use_lo=False, recip_fast=True, gps_cast=True):
    """Build + compile the single-core program (SPMD across 8 cores)."""
    s = seq_len
    n_stripes = s // STRIPE
    n_chunks = s // 128
    pdt = F8 if proj_fp8 else BF16
    psc = INV_WSCALE if proj_fp8 else 1.0

    nc = bacc.Bacc("TRN2", target_bir_lowering=False, debug=False, num_devices=8)

    x_d = nc.dram_tensor("x", [s, H], F32, kind="ExternalInput").ap()
    m_d = nc.dram_tensor("mask", [s], F32, kind="ExternalInput").ap()
    wq_d = nc.dram_tensor("wq", [H, H], F32, kind="ExternalInput").ap()
    wk_d = nc.dram_tensor("wk", [H, H], F32, kind="ExternalInput").ap()
    wv_d = nc.dram_tensor("wv", [H, H], F32, kind="ExternalInput").ap()
    if with_bias:
        bq_d = nc.dram_tensor("bq", [H], F32, kind="ExternalInput").ap()
        bk_d = nc.dram_tensor("bk", [H], F32, kind="ExternalInput").ap()
        bv_d = nc.dram_tensor("bv", [H], F32, kind="ExternalInput").ap()
    out_d = nc.dram_tensor("out", [s, H], F32, kind="ExternalOutput").ap()

    with tile.TileContext(nc) as tc:
        with ExitStack() as ctx:
            singles = ctx.enter_context(tc.tile_pool(name="singles", bufs=1))
            xpool = ctx.enter_context(tc.tile_pool(name="xpool", bufs=4))
            x8pool = ctx.enter_context(tc.tile_pool(name="x8pool", bufs=3))
            xtpool = ctx.enter_context(tc.tile_pool(name="xtpool", bufs=2))
            eqpool = ctx.enter_context(tc.tile_pool(name="eqpool", bufs=3))
            vapool = ctx.enter_context(tc.tile_pool(name="vapool", bufs=3))
            opool = ctx.enter_context(tc.tile_pool(name="opool", bufs=4))
            small = ctx.enter_context(tc.tile_pool(name="small", bufs=6))
            # PSUM (8 banks): tp 2 + proj 3 + p2 3
            tp = ctx.enter_context(tc.tile_pool(name="tp", bufs=2, space="PSUM"))
            proj = ctx.enter_context(tc.tile_pool(name="proj", bufs=3, space="PSUM"))
            p2p = ctx.enter_context(tc.tile_pool(name="p2p", bufs=1, space="PSUM"))

            ident = singles.tile([128, 128], F32)
            make_identity(nc, ident)
            identp = singles.tile([128, 128], pdt)
            nc.vector.tensor_copy(identp, ident)

            mask_sb = singles.tile([128, n_chunks], F32)
            nc.gpsimd.dma_start(out=mask_sb, in_=m_d.rearrange("(c p) -> p c", p=128))

            # weights: DMA fp32 staging chunks, scale+round to fp8 (or bf16)
            wsc = WSCALE if proj_fp8 else 1.0
            w_r = {}
            wstage = ctx.enter_context(tc.tile_pool(name="wstage", bufs=3))
            for name, wd in (("wq", wq_d), ("wv", wv_d), ("wk", wk_d)):
                wr = singles.tile([128, KT, H], pdt, tag=f"{name}_r", name=f"{name}_r")
                w_r[name] = wr
                for k in range(KT):
                    st = wstage.tile([128, H], F32, tag="wst", name="wst")
                    nc.gpsimd.dma_start(out=st, in_=wd[k * 128 : (k + 1) * 128, :])
                    if proj_fp8 or not gps_cast:
                        nc.scalar.activation(wr[:, k, :], st, COPYF, scale=wsc)
                    else:
                        nc.gpsimd.tensor_copy(wr[:, k, :], st)
            wq_r, wk_r, wv_r = w_r["wq"], w_r["wk"], w_r["wv"]

            if with_bias:
                # bq pre-scaled by wsc (added to scaled psum before exp);
                # bv added unscaled after the V evacuation descale.
                bqb = singles.tile([128, H], F32)
                bvb = singles.tile([128, H], F32)
                for bt, bd, bs in ((bqb, bq_d, wsc), (bvb, bv_d, 1.0)):
                    st = opool.tile([128, H], F32, tag="ob", name="bst")
                    src = bass.AP(
                        tensor=bd.tensor, offset=bd.offset, ap=[[0, 128], bd.ap[0]]
                    )
                    nc.sync.dma_start(out=st, in_=src)
                    nc.scalar.activation(bt, st, COPYF, scale=bs)
                bkc = singles.tile([128, KT], F32)
                nc.sync.dma_start(out=bkc, in_=bk_d.rearrange("(t p) -> p t", p=128))

            # EKT: exp(K^T) for the whole sequence, bf16, SBUF-resident
            ekt = singles.tile([128, KT, s], BF16)

            # s_ctx staging: [128, NP, 128]; cols 0:64 head-A sctx,
            # 64:128 head-B sctx (row blocks 0:64 / 64:128); off-diagonal
            # blocks stay zero.
            sctf = singles.tile([128, NP, 128], F32)
            sctx = singles.tile([128, NP, 128], BF16)
            if use_lo:
                sctl = singles.tile([128, NP, 128], BF16)
            nc.vector.memset(sctf, 0.0)

            # block-diagonal ones operand for the pass-B denominator matmuls
            # (partial-partition writes via tensor_copy from fp32 tiles;
            # partition-offset memset does not lower correctly)
            ones1 = singles.tile([128, 1], F32)
            nc.vector.memset(ones1, 1.0)
            z2 = singles.tile([128, 2], F32)
            nc.vector.memset(z2, 0.0)
            ones2 = singles.tile([128, 2], BF16)
            nc.vector.tensor_copy(ones2, z2)
            nc.vector.tensor_copy(ones2[0:64, 0:1], ones1[0:64, :])
            nc.vector.tensor_copy(ones2[64:128, 1:2], ones1[64:128, :])

            # phase-2 PSUM accumulators: 3 banks, head-pairs (0,1,2),
            # (3,4,5), (6,7) at 130-col offsets
            p2b = [
                p2p.tile([128, 390], F32, tag=f"p2b{i}", name=f"p2b{i}")
                for i in range(3)
            ]

            def p2ap(hp):
                return p2b[hp // 3][:, (hp % 3) * 130 : (hp % 3 + 1) * 130]

            P2_FIRST = {0, 3, 6}
            P2_LAST = {2, 5, 7}

            def emit_proj(pq, lhsT_tile, lhs_cs, w_tile, w_cs):
                """psum += xT_chunk.T @ W (or W.T @ xT for the K path)."""
                if proj_fp8:
                    for kp in range(KT // 2):
                        ks = slice(2 * kp, 2 * kp + 2)
                        nc.tensor.matmul(
                            pq,
                            lhsT_tile[:, ks, lhs_cs],
                            w_tile[:, ks, w_cs],
                            start=kp == 0,
                            stop=kp == KT // 2 - 1,
                            perf_mode=DR,
                        )
                else:
                    for k in range(KT):
                        nc.tensor.matmul(
                            pq,
                            lhsT_tile[:, k, lhs_cs],
                            w_tile[:, k, w_cs],
                            start=k == 0,
                            stop=k == KT - 1,
                        )

            loop_cm = tc.For_i(0, loop_n, 1) if loop_n else contextlib.nullcontext()
            with loop_cm:
                pend = None  # (eq_tile, va_tile, global_chunk) awaiting phase 2

                def emit_p2(eqc, vac, sc):
                    for hp in range(NP):
                        nc.tensor.matmul(
                            p2ap(hp),
                            eqc[:, hp * 128 : (hp + 1) * 128],
                            vac[:, hp * 2 : hp * 2 + 2, :],
                            start=(sc == 0 and hp in P2_FIRST),
                            stop=(sc == n_chunks - 1 and hp in P2_LAST),
                        )

                for st_i in range(n_stripes):
                    xt8 = xtpool.tile([128, KT, STRIPE], pdt, name="xt8")

                    def emit_t(c):
                        sc = st_i * CPS + c
                        xc = xpool.tile([128, H], F32, tag="xc", name="xc")
                        nc.sync.dma_start(
                            out=xc, in_=x_d[sc * 128 : (sc + 1) * 128, :]
                        )
                        xf8 = x8pool.tile([128, H], pdt, tag="xf8", name="xf8")
                        (nc.gpsimd if gps_cast else nc.vector).tensor_copy(xf8, xc)
                        for g in range(2):
                            pt = tp.tile([128, 4, 128], pdt, tag="pt", name="pt")
                            for j in range(4):
                                k = g * 4 + j
                                nc.tensor.transpose(
                                    pt[:, j, :],
                                    xf8[:, k * 128 : (k + 1) * 128],
                                    identp,
                                )
                            nc.vector.tensor_copy(
                                xt8[:, g * 4 : (g + 1) * 4, c * 128 : (c + 1) * 128],
                                pt,
                            )

                    emit_t(0)
                    for c in range(CPS):
                        sc = st_i * CPS + c
                        cs = slice(c * 128, (c + 1) * 128)
                        # Q projection -> exp -> EQ bf16
                        eqc = eqpool.tile([128, H], BF16, tag="eq", name="eq")
                        mb = mask_sb[:, sc : sc + 1]
                        for half in range(2):
                            hs = slice(half * 512, (half + 1) * 512)
                            pq = proj.tile([128, 512], F32, tag="proj", name="pq")
                            emit_proj(pq, xt8, cs, wq_r, hs)
                            if with_bias:
                                nc.vector.tensor_add(pq, pq, bqb[:, hs])
                            nc.scalar.activation(
                                eqc[:, hs], pq, EXPF, bias=mb, scale=psc
                            )
                        if c + 1 < CPS:
                            emit_t(c + 1)
                        # V projection -> descale -> VA bf16 (+ ones column)
                        vac = vapool.tile([128, NH, 65], BF16, tag="va", name="va")
                        for half in range(2):
                            hs = slice(half * 512, (half + 1) * 512)
                            pv = proj.tile([128, 512], F32, tag="proj", name="pv")
                            emit_proj(pv, xt8, cs, wv_r, hs)
                            dst = vac[:, half * 8 : (half + 1) * 8, 0:64]
                            src = pv[:].rearrange("p (h e) -> p h e", e=64)
                            if proj_fp8:
                                nc.vector.tensor_scalar_mul(dst, src, INV_WSCALE)
                            else:
                                nc.vector.tensor_copy(dst, src)
                            if with_bias:
                                nc.vector.tensor_add(
                                    dst,
                                    dst,
                                    bvb[:, hs].rearrange("p (h e) -> p h e", e=64),
                                )
                        nc.vector.memset(vac[:, :, 64:65], 1.0)
                        if pend is not None:
                            emit_p2(*pend)
                        pend = (eqc, vac, sc)

                    # K-transposed projection -> exp -> EKT (SBUF)
                    for t in range(KT):
                        ts_ = slice(t * 128, (t + 1) * 128)
                        pk = proj.tile([128, 512], F32, tag="proj", name="pk")
                        emit_proj(pk, wk_r, ts_, xt8, slice(0, STRIPE))
                        kb = {"bias": bkc[:, t : t + 1]} if with_bias else {}
                        nc.scalar.activation(
                            ekt[:, t, st_i * STRIPE : (st_i + 1) * STRIPE],
                            pk,
                            EXPF,
                            scale=psc,
                            **kb,
                        )

                # last pending phase 2
                emit_p2(*pend)
                pend = None

                # ---------------- finalize s_ctx ----------------
                for hp in range(NP):
                    pv2 = p2ap(hp)
                    r0 = small.tile([64, 1], F32, tag="r0", name="r0")
                    r1 = small.tile([64, 1], F32, tag="r1", name="r1")
                    nc.vector.reciprocal(r0, pv2[0:64, 64:65])
                    nc.vector.reciprocal(r1, pv2[64:128, 129:130])
                    nc.vector.tensor_scalar_mul(
                        sctf[0:64, hp, 0:64], pv2[0:64, 0:64], r0
                    )
                    nc.vector.tensor_scalar_mul(
                        sctf[64:128, hp, 64:128], pv2[64:128, 65:129], r1
                    )
                nc.vector.tensor_copy(sctx, sctf)
                if use_lo:
                    nc.vector.tensor_tensor(
                        out=sctl, in0=sctf, in1=sctx, op=mybir.AluOpType.subtract
                    )

                # ---------------- PASS B ----------------
                # Per chunk: 8 tiny matmuls EKT_hp.T @ ones2 fill a [128, 16]
                # denominator tile (one bank); main matmuls pack 4 head-pairs
                # per bank (4 x 128 cols); evacuation is one
                # reciprocal_approx_fast + two broadcast multiplies.
                for cc in range(n_chunks):
                    ob = opool.tile([128, H], F32, tag="ob", name="ob")
                    ccs = slice(cc * 128, (cc + 1) * 128)
                    pd = proj.tile([128, 512], F32, tag="proj", name="pd")
                    for hp in range(NP):
                        nc.tensor.matmul(
                            pd[:, hp * 2 : hp * 2 + 2],
                            ekt[:, hp, ccs],
                            ones2,
                            start=hp == 0,
                            stop=hp == NP - 1,
                        )
                    r16 = small.tile([128, 16], F32, tag="r16", name="r16")
                    recip = (nc.vector.reciprocal_approx_fast if recip_fast
                             else nc.vector.reciprocal)
                    recip(r16, pd[:, 0:16])
                    for g in range(2):
                        p3 = proj.tile([128, 512], F32, tag="proj", name="p3")
                        for i in range(4):
                            hp = g * 4 + i
                            nc.tensor.matmul(
                                p3[:, i * 128 : (i + 1) * 128],
                                ekt[:, hp, ccs],
                                sctx[:, hp, :],
                                start=(i == 0),
                                stop=(i == 3 and not use_lo),
                            )
                        if use_lo:
                            for i in range(4):
                                hp = g * 4 + i
                                nc.tensor.matmul(
                                    p3[:, i * 128 : (i + 1) * 128],
                                    ekt[:, hp, ccs],
                                    sctl[:, hp, :],
                                    start=False,
                                    stop=(i == 3),
                                )
                        obv = ob[:, g * 512 : (g + 1) * 512].rearrange(
                            "p (h e) -> p h e", e=64
                        )
                        p3v = p3[:].rearrange("p (h e) -> p h e", e=64)
                        nc.vector.tensor_tensor(
                            out=obv,
                            in0=p3v,
                            in1=_bcast(r16[:, g * 8 : (g + 1) * 8], 64),
                            op=mybir.AluOpType.mult,
                        )
                    eng = nc.gpsimd if cc % 2 == 0 else nc.sync
                    eng.dma_start(out=out_d[ccs, :], in_=ob)

    nc.compile()
    return nc


_CACHE = {}


def _get_nc(seq_len, with_bias):
    key = (seq_len, with_bias)
    if key not in _CACHE:
        _CACHE[key] = build_kernel(seq_len, with_bias)
    return _CACHE[key]


def kernel(hidden_states, attention_mask, Wq, bq, Wk, bk, Wv, bv):
    hidden_states = np.asarray(hidden_states, dtype=np.float32)
    attention_mask = np.asarray(attention_mask, dtype=np.float32)
    Wq = np.asarray(Wq, dtype=np.float32)
    Wk = np.asarray(Wk, dtype=np.float32)
    Wv = np.asarray(Wv, dtype=np.float32)
    bq = np.asarray(bq, dtype=np.float32)
    bk = np.asarray(bk, dtype=np.float32)
    bv = np.asarray(bv, dtype=np.float32)
    b, s, h = hidden_states.shape
    with_bias = bool(bq.any() or bk.any() or bv.any())
    nc = _get_nc(s, with_bias)

    mask = attention_mask.reshape(b, s)
    in_maps = []
    for i in range(b):
        m = {
            "x": np.ascontiguousarray(hidden_states[i]),
            "mask": np.ascontiguousarray(mask[i]),
            "wq": Wq,
            "wk": Wk,
            "wv": Wv,
        }
        if with_bias:
            m.update({"bq": bq, "bk": bk, "bv": bv})
        in_maps.append(m)

    res = bass_utils.run_bass_kernel_spmd(nc, in_maps, core_ids=list(range(b)))
    return np.stack([res.results[i]["out"] for i in range(b)], axis=0)


# revision 17
# speedup vs baseline: 1.7656x; 1.0278x over previous
"""Trainium2 Bass kernel for BERT factorized attention.

Reference math (per batch b, head h, S=4096, H=1024, NH=16, HD=64):
    q = x @ Wq + bq ; k = x @ Wk + bk ; v = x @ Wv + bv
    s_probs = softmax_S(qT_head)            # [HD, S]
    c_probs = softmax_HD(k_head)            # [S, HD]
    s_ctx   = s_probs @ v_head              # [HD, HD]
    out     = c_probs @ s_ctx               # [S, HD]

Kernel strategy (one batch element per NeuronCore, 8 cores, no collectives):
  - X cast to fp8e4m3, PE-transposed per chunk; Q/K/V projections run as
    fp8 DoubleRow matmuls (256-deep contraction, 2x PE rate). Weights are
    pre-scaled by 32 into fp8; the scale is undone inside the exp
    activation (scale=1/32) / V evacuation.
  - softmax-over-S denominators come from an augmented ones column in the
    V operand of phase 2 (s_ctx accumulation); phase 2 accumulates
    directly in PSUM across all 32 chunks (3 pinned banks, one
    accumulation group per bank).
  - EKT (exp of K-transposed, bf16) lives entirely in SBUF.
  - softmax-over-HD denominators come from two ones-columns appended to
    s_ctx in the pass-B matmul; out = p3 * recip(den) per head via
    fused tensor_scalar/activation-scale evacuations.
"""

import sys

sys.path.insert(0, "/opt/trn_rl_repo")

import contextlib
from contextlib import ExitStack

import numpy as np

import concourse.bass as bass
import concourse.mybir as mybir
import concourse.tile as tile
from concourse import bacc, bass_utils
from concourse.masks import make_identity

F32 = mybir.dt.float32
BF16 = mybir.dt.bfloat16
F8 = mybir.dt.float8e4

B, S, H = 8, 4096, 1024
NH, HD = 16, 64
STRIPE = 512
CPS = STRIPE // 128  # chunks per stripe
KT = H // 128  # contraction tiles
NP = NH // 2  # head pairs

WSCALE = 32.0
INV_WSCALE = 1.0 / WSCALE

EXPF = mybir.ActivationFunctionType.Exp
COPYF = mybir.ActivationFunctionType.Copy
DR = mybir.MatmulPerfMode.DoubleRow


def _bcast(ap_2d, n):
    """[p, c] AP -> [p, c, n] with step-0 broadcast on the last dim."""
    return bass.AP(
        tensor=ap_2d.tensor,
        offset=ap_2d.offset,
        ap=[ap_2d.ap[0], ap_2d.ap[1], [0, n]],
    )


def build_kernel(seq_len=S, with_bias=False, loop_n=None, proj_fp8=False,
                 use_lo=False, recip_fast=True, gps_cast=True):
    """Build + compile the single-core program (SPMD across 8 cores)."""
    s = seq_len
    n_stripes = s // STRIPE
    n_chunks = s // 128
    pdt = F8 if proj_fp8 else BF16
    psc = INV_WSCALE if proj_fp8 else 1.0

    nc = bacc.Bacc("TRN2", target_bir_lowering=False, debug=False, num_devices=8)

    x_d = nc.dram_tensor("x", [s, H], F32, kind="ExternalInput").ap()
    m_d = nc.dram_tensor("mask", [s], F32, kind="ExternalInput").ap()
    wq_d = nc.dram_tensor("wq", [H, H], F32, kind="ExternalInput").ap()
    wk_d = nc.dram_tensor("wk", [H, H], F32, kind="ExternalInput").ap()
    wv_d = nc.dram_tensor("wv", [H, H], F32, kind="ExternalInput").ap()
    if with_bias:
        bq_d = nc.dram_tensor("bq", [H], F32, kind="ExternalInput").ap()
        bk_d = nc.dram_tensor("bk", [H], F32, kind="ExternalInput").ap()
        bv_d = nc.dram_tensor("bv", [H], F32, kind="ExternalInput").ap()
    out_d = nc.dram_tensor("out", [s, H], F32, kind="ExternalOutput").ap()

    with tile.TileContext(nc) as tc:
        with ExitStack() as ctx:
            singles = ctx.enter_context(tc.tile_pool(name="singles", bufs=1))
            xpool = ctx.enter_context(tc.tile_pool(name="xpool", bufs=4))
            x8pool = ctx.enter_context(tc.tile_pool(name="x8pool", bufs=3))
            xtpool = ctx.enter_context(tc.tile_pool(name="xtpool", bufs=2))
            eqpool = ctx.enter_context(tc.tile_pool(name="eqpool", bufs=3))
            vapool = ctx.enter_context(tc.tile_pool(name="vapool", bufs=3))
            opool = ctx.enter_context(tc.tile_pool(name="opool", bufs=4))
            small = ctx.enter_context(tc.tile_pool(name="small", bufs=6))
            # PSUM (8 banks): tp 2 + proj 3 + p2 3
            tp = ctx.enter_context(tc.tile_pool(name="tp", bufs=2, space="PSUM"))
            proj = ctx.enter_context(tc.tile_pool(name="proj", bufs=3, space="PSUM"))
            p2p = ctx.enter_context(tc.tile_pool(name="p2p", bufs=1, space="PSUM"))

            ident = singles.tile([128, 128], F32)
            make_identity(nc, ident)
            identp = singles.tile([128, 128], pdt)
            nc.vector.tensor_copy(identp, ident)

            mask_sb = singles.tile([128, n_chunks], F32)
            nc.gpsimd.dma_start(out=mask_sb, in_=m_d.rearrange("(c p) -> p c", p=128))

            # weights: DMA fp32 staging chunks, scale+round to fp8 (or bf16)
            wsc = WSCALE if proj_fp8 else 1.0
            w_r = {}
            wstage = ctx.enter_context(tc.tile_pool(name="wstage", bufs=3))
            for name, wd in (("wq", wq_d), ("wv", wv_d), ("wk", wk_d)):
                wr = singles.tile([128, KT, H], pdt, tag=f"{name}_r", name=f"{name}_r")
                w_r[name] = wr
                for k in range(KT):
                    st = wstage.tile([128, H], F32, tag="wst", name="wst")
                    nc.gpsimd.dma_start(out=st, in_=wd[k * 128 : (k + 1) * 128, :])
                    if proj_fp8 or not gps_cast:
                        nc.scalar.activation(wr[:, k, :], st, COPYF, scale=wsc)
                    else:
                        nc.gpsimd.tensor_copy(wr[:, k, :], st)
            wq_r, wk_r, wv_r = w_r["wq"], w_r["wk"], w_r["wv"]

            if with_bias:
                # bq pre-scaled by wsc (added to scaled psum before exp);
                # bv added unscaled after the V evacuation descale.
                bqb = singles.tile([128, H], F32)
                bvb = singles.tile([128, H], F32)
                for bt, bd, bs in ((bqb, bq_d, wsc), (bvb, bv_d, 1.0)):
                    st = opool.tile([128, H], F32, tag="ob", name="bst")
                    src = bass.AP(
                        tensor=bd.tensor, offset=bd.offset, ap=[[0, 128], bd.ap[0]]
                    )
                    nc.sync.dma_start(out=st, in_=src)
                    nc.scalar.activation(bt, st, COPYF, scale=bs)
                bkc = singles.tile([128, KT], F32)
                nc.sync.dma_start(out=bkc, in_=bk_d.rearrange("(t p) -> p t", p=128))

            # EKT: exp(K^T) for the whole sequence, bf16, SBUF-resident
            ekt = singles.tile([128, KT, s], BF16)

            # s_ctx staging: [128, NP, 128]; cols 0:64 head-A sctx,
            # 64:128 head-B sctx (row blocks 0:64 / 64:128); off-diagonal
            # blocks stay zero.
            sctf = singles.tile([128, NP, 128], F32)
            sctx = singles.tile([128, NP, 128], BF16)
            if use_lo:
                sctl = singles.tile([128, NP, 128], BF16)
            nc.vector.memset(sctf, 0.0)

            # block-diagonal ones operand for the pass-B denominator matmuls
            # (partial-partition writes via tensor_copy from fp32 tiles;
            # partition-offset memset does not lower correctly)
            ones1 = singles.tile([128, 1], F32)
            nc.vector.memset(ones1, 1.0)
            z2 = singles.tile([128, 2], F32)
            nc.vector.memset(z2, 0.0)
            ones2 = singles.tile([128, 2], BF16)
            nc.vector.tensor_copy(ones2, z2)
            nc.vector.tensor_copy(ones2[0:64, 0:1], ones1[0:64, :])
            nc.vector.tensor_copy(ones2[64:128, 1:2], ones1[64:128, :])

            # phase-2 PSUM accumulators: 3 banks, head-pairs (0,1,2),
            # (3,4,5), (6,7) at 130-col offsets
            p2b = [
                p2p.tile([128, 390], F32, tag=f"p2b{i}", name=f"p2b{i}")
                for i in range(3)
            ]

            def p2ap(hp):
                return p2b[hp // 3][:, (hp % 3) * 130 : (hp % 3 + 1) * 130]

            P2_FIRST = {0, 3, 6}
            P2_LAST = {2, 5, 7}

            def emit_proj(pq, lhsT_tile, lhs_cs, w_tile, w_cs):
                """psum += xT_chunk.T @ W (or W.T @ xT for the K path)."""
                if proj_fp8:
                    for kp in range(KT // 2):
                        ks = slice(2 * kp, 2 * kp + 2)
                        nc.tensor.matmul(
                            pq,
                            lhsT_tile[:, ks, lhs_cs],
                            w_tile[:, ks, w_cs],
                            start=kp == 0,
                            stop=kp == KT // 2 - 1,
                            perf_mode=DR,
                        )
                else:
                    for k in range(KT):
                        nc.tensor.matmul(
                            pq,
                            lhsT_tile[:, k, lhs_cs],
                            w_tile[:, k, w_cs],
                            start=k == 0,
                            stop=k == KT - 1,
                        )

            loop_cm = tc.For_i(0, loop_n, 1) if loop_n else contextlib.nullcontext()
            with loop_cm:
                pend = None  # (eq_tile, va_tile, global_chunk) awaiting phase 2

                def emit_p2(eqc, vac, sc):
                    for hp in range(NP):
                        nc.tensor.matmul(
                            p2ap(hp),
                            eqc[:, hp * 128 : (hp + 1) * 128],
                            vac[:, hp * 2 : hp * 2 + 2, :],
                            start=(sc == 0 and hp in P2_FIRST),
                            stop=(sc == n_chunks - 1 and hp in P2_LAST),
                        )

                for st_i in range(n_stripes):
                    xt8 = xtpool.tile([128, KT, STRIPE], pdt, name="xt8")

                    def emit_t(c):
                        sc = st_i * CPS + c
                        xc = xpool.tile([128, H], F32, tag="xc", name="xc")
                        nc.sync.dma_start(
                            out=xc, in_=x_d[sc * 128 : (sc + 1) * 128, :]
                        )
                        xf8 = x8pool.tile([128, H], pdt, tag="xf8", name="xf8")
                        (nc.gpsimd if gps_cast else nc.vector).tensor_copy(xf8, xc)
                        for g in range(2):
                            pt = tp.tile([128, 4, 128], pdt, tag="pt", name="pt")
                            for j in range(4):
                                k = g * 4 + j
                                nc.tensor.transpose(
                                    pt[:, j, :],
                                    xf8[:, k * 128 : (k + 1) * 128],
                                    identp,
                                )
                            nc.vector.tensor_copy(
                                xt8[:, g * 4 : (g + 1) * 4, c * 128 : (c + 1) * 128],
                                pt,
                            )

                    emit_t(0)
                    for c in range(CPS):
                        sc = st_i * CPS + c
                        cs = slice(c * 128, (c + 1) * 128)
                        # Q projection -> exp -> EQ bf16
                        eqc = eqpool.tile([128, H], BF16, tag="eq", name="eq")
                        mb = mask_sb[:, sc : sc + 1]
                        for half in range(2):
                            hs = slice(half * 512, (half + 1) * 512)
                            pq = proj.tile([128, 512], F32, tag="proj", name="pq")
                            emit_proj(pq, xt8, cs, wq_r, hs)
                            if with_bias:
                                nc.vector.tensor_add(pq, pq, bqb[:, hs])
                            nc.scalar.activation(
                                eqc[:, hs], pq, EXPF, bias=mb, scale=psc
                            )
                        if c + 1 < CPS:
                            emit_t(c + 1)
                        # V projection -> descale -> VA bf16 (+ ones column)
                        vac = vapool.tile([128, NH, 65], BF16, tag="va", name="va")
                        for half in range(2):
                            hs = slice(half * 512, (half + 1) * 512)
                            pv = proj.tile([128, 512], F32, tag="proj", name="pv")
                            emit_proj(pv, xt8, cs, wv_r, hs)
                            dst = vac[:, half * 8 : (half + 1) * 8, 0:64]
                            src = pv[:].rearrange("p (h e) -> p h e", e=64)
                            if proj_fp8:
                                nc.vector.tensor_scalar_mul(dst, src, INV_WSCALE)
                            else:
                                nc.vector.tensor_copy(dst, src)
                            if with_bias:
                                nc.vector.tensor_add(
                                    dst,
                                    dst,
                                    bvb[:, hs].rearrange("p (h e) -> p h e", e=64),
                                )
                        nc.vector.memset(vac[:, :, 64:65], 1.0)
                        if pend is not None:
                            emit_p2(*pend)
                        pend = (eqc, vac, sc)

                    # K-transposed projection -> exp -> EKT (SBUF)
                    for t in range(KT):
                        ts_ = slice(t * 128, (t + 1) * 128)
                        pk = proj.tile([128, 512], F32, tag="proj", name="pk")
                        emit_proj(pk, wk_r, ts_, xt8, slice(0, STRIPE))
                        kb = {"bias": bkc[:, t : t + 1]} if with_bias else {}
                        nc.scalar.activation(
                            ekt[:, t, st_i * STRIPE : (st_i + 1) * STRIPE],
                            pk,
                            EXPF,
                            scale=psc,
                            **kb,
                        )

                # last pending phase 2
                emit_p2(*pend)
                pend = None

                # ---------------- finalize s_ctx ----------------
                for hp in range(NP):
                    pv2 = p2ap(hp)
                    r0 = small.tile([64, 1], F32, tag="r0", name="r0")
                    r1 = small.tile([64, 1], F32, tag="r1", name="r1")
                    nc.vector.reciprocal(r0, pv2[0:64, 64:65])
                    nc.vector.reciprocal(r1, pv2[64:128, 129:130])
                    nc.vector.tensor_scalar_mul(
                        sctf[0:64, hp, 0:64], pv2[0:64, 0:64], r0
                    )
                    nc.vector.tensor_scalar_mul(
                        sctf[64:128, hp, 64:128], pv2[64:128, 65:129], r1
                    )
                nc.vector.tensor_copy(sctx, sctf)
                if use_lo:
                    nc.vector.tensor_tensor(
                        out=sctl, in0=sctf, in1=sctx, op=mybir.AluOpType.subtract
                    )

                # ---------------- PASS B ----------------
                # Per chunk: 8 tiny matmuls EKT_hp.T @ ones2 fill a [128, 16]
                # denominator tile (one bank); main matmuls pack 4 head-pairs
                # per bank (4 x 128 cols); evacuation is one
                # reciprocal_approx_fast + two broadcast multiplies.
                recip = (nc.vector.reciprocal_approx_fast if recip_fast
                         else nc.vector.reciprocal)
                r16s = {}

                def emit_den(cc):
                    """Denominator matmuls + reciprocal, one chunk ahead of
                    the main pass-B matmuls so the DVE chain overlaps PE."""
                    ccs = slice(cc * 128, (cc + 1) * 128)
                    pd = proj.tile([128, 512], F32, tag="proj", name="pd")
                    for hp in range(NP):
                        nc.tensor.matmul(
                            pd[:, hp * 2 : hp * 2 + 2],
                            ekt[:, hp, ccs],
                            ones2,
                            start=hp == 0,
                            stop=hp == NP - 1,
                        )
                    r16 = small.tile([128, 16], F32, tag="r16", name="r16")
                    recip(r16, pd[:, 0:16])
                    r16s[cc] = r16

                emit_den(0)
                for cc in range(n_chunks):
                    if cc + 1 < n_chunks:
                        emit_den(cc + 1)
                    r16 = r16s.pop(cc)
                    ob = opool.tile([128, H], F32, tag="ob", name="ob")
                    ccs = slice(cc * 128, (cc + 1) * 128)
                    for g in range(2):
                        p3 = proj.tile([128, 512], F32, tag="proj", name="p3")
                        for i in range(4):
                            hp = g * 4 + i
                            nc.tensor.matmul(
                                p3[:, i * 128 : (i + 1) * 128],
                                ekt[:, hp, ccs],
                                sctx[:, hp, :],
                                start=(i == 0),
                                stop=(i == 3 and not use_lo),
                            )
                        if use_lo:
                            for i in range(4):
                                hp = g * 4 + i
                                nc.tensor.matmul(
                                    p3[:, i * 128 : (i + 1) * 128],
                                    ekt[:, hp, ccs],
                                    sctl[:, hp, :],
                                    start=False,
                                    stop=(i == 3),
                                )
                        obv = ob[:, g * 512 : (g + 1) * 512].rearrange(
                            "p (h e) -> p h e", e=64
                        )
                        p3v = p3[:].rearrange("p (h e) -> p h e", e=64)
                        nc.vector.tensor_tensor(
                            out=obv,
                            in0=p3v,
                            in1=_bcast(r16[:, g * 8 : (g + 1) * 8], 64),
                            op=mybir.AluOpType.mult,
                        )
                    eng = nc.gpsimd if cc % 2 == 0 else nc.sync
                    eng.dma_start(out=out_d[ccs, :], in_=ob)

    nc.compile()
    return nc


_CACHE = {}


def _get_nc(seq_len, with_bias):
    key = (seq_len, with_bias)
    if key not in _CACHE:
        _CACHE[key] = build_kernel(seq_len, with_bias)
    return _CACHE[key]


def kernel(hidden_states, attention_mask, Wq, bq, Wk, bk, Wv, bv):
    hidden_states = np.asarray(hidden_states, dtype=np.float32)
    attention_mask = np.asarray(attention_mask, dtype=np.float32)
    Wq = np.asarray(Wq, dtype=np.float32)
    Wk = np.asarray(Wk, dtype=np.float32)
    Wv = np.asarray(Wv, dtype=np.float32)
    bq = np.asarray(bq, dtype=np.float32)
    bk = np.asarray(bk, dtype=np.float32)
    bv = np.asarray(bv, dtype=np.float32)
    b, s, h = hidden_states.shape
    with_bias = bool(bq.any() or bk.any() or bv.any())
    nc = _get_nc(s, with_bias)

    mask = attention_mask.reshape(b, s)
    in_maps = []
    for i in range(b):
        m = {
            "x": np.ascontiguousarray(hidden_states[i]),
            "mask": np.ascontiguousarray(mask[i]),
            "wq": Wq,
            "wk": Wk,
            "wv": Wv,
        }
        if with_bias:
            m.update({"bq": bq, "bk": bk, "bv": bv})
        in_maps.append(m)

    res = bass_utils.run_bass_kernel_spmd(nc, in_maps, core_ids=list(range(b)))
    return np.stack([res.results[i]["out"] for i in range(b)], axis=0)
